# revision 36
# baseline (speedup 1.0000x reference)
"""Trainium2 Bass kernel for CustomSNNLoss (nn_CustomSNNLoss_36429912604816).

Strategy (data-parallel over rows of the NxN similarity):
  - Host: normalize x, round to fp8e4 (cosine-sim errors are unbiased and
    average out over the ~300-6000 element class sums), build per-key-pair
    one-hot matrices (combo 112-pad / target 32-pad for DoubleRow).
  - Each of the 8 cores owns R = 768 query rows, split into chunks of
    A=512 / B=256 queries (PSUM-bank-exact matmul outputs). Each core
    receives xnt ROTATED so its own queries sit at columns 0:R (the query
    operand is a slice of the key operand - no separate xnq transfer).
    Keys are processed in 24 pairs of 128-key tiles:
        sim tiles     : plain fp8 matmuls into paired PSUM tiles [128, 2F]
        Sb_A = exp(2s): one ScalarE activation per paired tile, fp8 out
        Sb_B          : VectorE Schraudolph (e4m3 bits = round(c1*s + 56))
        St = Sb^2     : VectorE int16 trick on the fp8 bit patterns
                        (bits(v^2) = 2*bits(v) - 56 for e4m3)
        class sums    : fp8 DoubleRow matmuls (2 key tiles per matmul at
                        0.5 cycles/row), software-pipelined one pair behind
                        the sims so the PE queue never blocks on exp.
    The NxN matrix never touches HBM.
  - Host epilogue (O(N)): per-row pos/neg sums from the class sums, -log
    losses, validity, class-weighted means. The device's fp8-rounded
    diagonal contribution is replicated exactly on the host (per-chunk:
    ScalarE-exp rounding for chunk A, Schraudolph bits for chunk B).
"""

import os
import numpy as np

N, D = 6144, 128
P = 128                 # partitions / key tile
NCORES = 8
R = N // NCORES         # 768 query rows per core
QA, QB = 512, 256       # query chunks (bank-exact PSUM outputs)
KT = N // P             # 48 key tiles
NPAIR = KT // 2         # 24 key-tile pairs
NT, NB = 20, 5          # target classes, batch keys
NCC = NT * NB           # 100 combined classes
OUT_ROWS = NCC + NT     # 120 = combo(100) | target(20)
WC = NCC + NT           # combined one-hot width
MIN_T, MAX_T = 0.1, 1.0
TEMP_BATCH = 0.5
EPS = 1e-8

_compile_cache = {}
LAST_RESULT = None  # BassKernelResults from the most recent device run


def _patch_ldw_opt():
    """walrus's LDW optimization rejects bass-emitted InstLdweights
    ("not compatible with LDW optimization"), so this stays opt-in for
    experiments only (KERNEL_LDW_OPT=1)."""
    if not os.environ.get("KERNEL_LDW_OPT"):
        return
    import concourse.bass_utils as bu
    if getattr(bu, "_ldw_opt_patched", False):
        return
    orig = bu.run_command

    def run_command_ldw(argv, **kwargs):
        argv = ["--enable-ldw-opt=true" if a == "--enable-ldw-opt=false"
                else a for a in argv]
        return orig(argv, **kwargs)

    bu.run_command = run_command_ldw
    bu._ldw_opt_patched = True


def _build(scale_t: float, scale_b: float, square_mode: bool, bias_t: float):
    from contextlib import ExitStack

    import concourse.bacc as bacc
    import concourse.mybir as mybir
    import concourse.tile as tile

    f32 = mybir.dt.float32
    f8 = mybir.dt.float8e4
    i16 = mybir.dt.int16
    i8 = mybir.dt.int8
    EXP = mybir.ActivationFunctionType.Exp
    DR = mybir.MatmulPerfMode.DoubleRow
    ADD = mybir.AluOpType.add
    MULT = mybir.AluOpType.mult

    nc = bacc.Bacc("TRN2", target_bir_lowering=False, debug=False,
                   enable_asserts=False)

    xnt = nc.dram_tensor("xnt", [P, N], f8, kind="ExternalInput").ap()
    wcp = nc.dram_tensor("wcp", [P, NPAIR * 2 * WC], f8,
                         kind="ExternalInput").ap()
    out = nc.dram_tensor("out", [OUT_ROWS, R], f32, kind="ExternalOutput").ap()

    with tile.TileContext(nc) as tc, ExitStack() as ctx:
        const = ctx.enter_context(tc.tile_pool(name="const", bufs=1))
        work = ctx.enter_context(tc.tile_pool(name="work", bufs=4))
        psim = ctx.enter_context(tc.tile_pool(name="psim", bufs=1, space="PSUM"))
        pacc = ctx.enter_context(tc.tile_pool(name="pacc", bufs=1, space="PSUM"))

        xnt_sb = const.tile([P, N], f8, name="xnt_sb")
        wcp_sb = const.tile([P, NPAIR * 2 * WC], f8, name="wcp_sb")
        out_sb = const.tile([OUT_ROWS, R], f32, name="out_sb")

        # Issue order: first sim needs xnq + xnt[:, :128]; first accs need
        # wcp chunk 0. Later chunks stream in under compute.
        nc.sync.dma_start(xnt_sb[:, 0:512], xnt[:, 0:512])
        nc.sync.dma_start(wcp_sb[:, 0:256], wcp[:, 0:256])
        nc.sync.dma_start(xnt_sb[:, 512:1024], xnt[:, 512:1024])
        nc.sync.dma_start(wcp_sb[:, 256:1024], wcp[:, 256:1024])
        nc.sync.dma_start(xnt_sb[:, 1024:2560], xnt[:, 1024:2560])
        nc.sync.dma_start(wcp_sb[:, 1024:3072], wcp[:, 1024:3072])
        nc.sync.dma_start(xnt_sb[:, 2560:4352], xnt[:, 2560:4352])
        nc.sync.dma_start(wcp_sb[:, 3072:6144], wcp[:, 3072:6144])
        nc.sync.dma_start(xnt_sb[:, 4352:6144], xnt[:, 4352:6144])

        accSb = pacc.tile([OUT_ROWS, QA], f32, name="accSb")
        accSt = pacc.tile([OUT_ROWS, QA], f32, name="accSt")
        accB = pacc.tile([OUT_ROWS, 2 * QB], f32, name="accB")

        def emit_acc_first(pend):
            sbA, stA, bt, wc_m, st, sp = pend
            nc.tensor.matmul(
                accSb[:], wc_m, sbA[:].rearrange("p (r q) -> p r q", r=2),
                start=st, stop=sp, perf_mode=DR)

        def emit_acc_rest(pend):
            sbA, stA, bt, wc_m, st, sp = pend
            nc.tensor.matmul(
                accSt[:], wc_m, stA[:].rearrange("p (r q) -> p r q", r=2),
                start=st, stop=sp, perf_mode=DR)
            nc.tensor.matmul(
                accB[:], wc_m, bt[:],
                start=st, stop=sp, perf_mode=DR)

        def emit_accs(pend):
            emit_acc_first(pend)
            emit_acc_rest(pend)

        pendings = []
        for m in range(NPAIR):
            pairA = psim.tile([P, 2 * QA], f32, tag="pairA", bufs=2,
                              name="pairA")
            pairB = psim.tile([P, 2 * QB], f32, tag="pairB", bufs=1,
                              name="pairB")
            # A-chunk sims first so the big ScalarE activation can start
            # as early as possible; B-chunk sims follow. The rotated xnt
            # doubles as the query operand (cols 0:R).
            for r in range(2):
                ksl = slice((2 * m + r) * P, (2 * m + r + 1) * P)
                nc.tensor.matmul(
                    pairA[:, r * QA:(r + 1) * QA],
                    xnt_sb[:, ksl],
                    xnt_sb[:, 0:QA],
                    start=True,
                    stop=True,
                )
            # software pipeline, depth 2: a pair's class-sum matmuls are
            # emitted two iterations later (first one here so its LDWEIGHTS
            # hides under the long A-sim streams), so their exp/square
            # inputs are long finished and the PE queue never blocks.
            ready = pendings.pop(0) if len(pendings) == 2 else None
            if ready is not None:
                emit_acc_first(ready)

            for r in range(2):
                ksl = slice((2 * m + r) * P, (2 * m + r + 1) * P)
                nc.tensor.matmul(
                    pairB[:, r * QB:(r + 1) * QB],
                    xnt_sb[:, ksl],
                    xnt_sb[:, QA:R],
                    start=True,
                    stop=True,
                )
            if ready is not None:
                emit_acc_rest(ready)


            sbA = work.tile([P, 2 * QA], f8, tag="sbA", name="sbA")
            # B-chunk products share one tile [p, r, Sb(256)|St(256)] so a
            # single DoubleRow matmul accumulates both against the combined
            # one-hot (combo rows 0:100 read from the Sb half, target rows
            # 100:120 from the St half).
            bt = work.tile([P, 2, 2 * QB], f8, tag="bt", name="bt")
            nc.scalar.activation(sbA[:], pairA[:], EXP, scale=scale_b)
            if square_mode:
                # Schraudolph exp on VectorE for the small chunk: the e4m3
                # bit pattern of exp(scale_b*s) is round(8*scale_b*log2(e)*s
                # + 56) (piecewise-linear log; unbiased, averages out in the
                # class sums). Frees ScalarE, the bottleneck engine.
                c1 = float(8.0 * scale_b * np.log2(np.e))
                nc.vector.tensor_scalar(
                    bt[:, :, 0:QB].bitcast(i8),
                    pairB[:].rearrange("p (r q) -> p r q", r=2),
                    c1, 56.0, MULT, ADD)
            else:
                nc.scalar.activation(
                    bt[:, :, 0:QB],
                    pairB[:].rearrange("p (r q) -> p r q", r=2),
                    EXP, scale=scale_b)

            stA = work.tile([P, 2 * QA], f8, tag="stA", name="stA")
            if square_mode:
                # e4m3 bit trick: bits(v^2) = 2*bits(v) - 56, carried out on
                # int16 pairs ((x - 0x1C1C) * 2 keeps both bytes carry-free).
                nc.vector.tensor_scalar(
                    stA[:].bitcast(i16), sbA[:].bitcast(i16),
                    -7196, 2, ADD, MULT)
                nc.vector.tensor_scalar(
                    bt[:, :, QB:2 * QB].bitcast(i16),
                    bt[:, :, 0:QB].bitcast(i16),
                    -7196, 2, ADD, MULT)
            else:
                nc.scalar.activation(stA[:], pairA[:], EXP, scale=scale_t,
                                     bias=bias_t)
                nc.scalar.activation(
                    bt[:, :, QB:2 * QB],
                    pairB[:].rearrange("p (r q) -> p r q", r=2),
                    EXP, scale=scale_t, bias=bias_t)

            wc_m = wcp_sb[:, m * 2 * WC:(m + 1) * 2 * WC].rearrange(
                "p (r c) -> p r c", r=2)
            pendings.append((sbA, stA, bt, wc_m,
                             m == 0, m == NPAIR - 1))
        for pend in pendings:
            emit_accs(pend)

        # out rows 0:100 = combo sums of Sb, rows 100:120 = target sums of
        # St. PSUM reads must start 32-aligned, so the St copies read rows
        # 96:120 and the Sb copies then overwrite rows 96:100 (ordered by
        # the overlapping-write dependency).
        nc.vector.tensor_copy(out_sb[96:OUT_ROWS, 0:QA],
                              accSt[96:OUT_ROWS, :])
        nc.vector.tensor_copy(out_sb[96:OUT_ROWS, QA:R],
                              accB[96:OUT_ROWS, QB:2 * QB])
        nc.scalar.copy(out_sb[0:NCC, 0:QA], accSb[0:NCC, :])
        nc.scalar.copy(out_sb[0:NCC, QA:R], accB[0:NCC, 0:QB])
        nc.gpsimd.dma_start(out[:], out_sb[:])

    nc.compile()
    return nc


def _get_compiled(scale_t: float, scale_b: float, square_mode: bool,
                  bias_t: float):
    key = (round(scale_t, 9), round(scale_b, 9), square_mode,
           round(bias_t, 9))
    if key not in _compile_cache:
        _compile_cache[key] = _build(scale_t, scale_b, square_mode, bias_t)
    return _compile_cache[key]


def _e4m3(v):
    import ml_dtypes
    return np.asarray(v).astype(ml_dtypes.float8_e4m3fn)


def _bitdouble_f64(v_fp8):
    """Replicate the device's e4m3 bit-doubling square on the host."""
    import ml_dtypes
    b = v_fp8.view(np.uint8).astype(np.int16)
    return ((b * 2 - 56).astype(np.uint8)
            .view(ml_dtypes.float8_e4m3fn).astype(np.float64))


def _host_prep(input, temperature, targets, batch0):
    x = np.asarray(input, dtype=np.float32)
    t = float(np.clip(np.float32(temperature), MIN_T, MAX_T))
    scale_t = 1.0 / t
    scale_b = 1.0 / TEMP_BATCH
    square_mode = abs(scale_t - 2.0 * scale_b) < 1e-6
    # General-t fallback keeps exp(scale_t*s + bias_t) inside fp8 range;
    # the loss is invariant to this uniform scale (host diag uses it too).
    bias_t = min(0.0, 6.0 - scale_t)

    norms = np.sqrt((x * x).sum(axis=1, keepdims=True, dtype=np.float32))
    norms = np.maximum(norms, np.float32(EPS)).astype(np.float32)
    xn8 = _e4m3((x / norms).astype(np.float32))
    xnf = xn8.astype(np.float32)
    s_ii = (xnf * xnf).sum(axis=1, dtype=np.float32)
    # [128, N] transposed embeddings; each core gets it rotated so its own
    # queries sit at columns 0:R.
    x3 = np.ascontiguousarray(xnf.T.astype(xn8.dtype))

    tg = np.asarray(targets).astype(np.int64)
    bt = np.asarray(batch0).astype(np.int64)
    combo = tg * NB + bt

    j = np.arange(N)
    wc48 = np.zeros((P, KT, WC), dtype=np.float32)
    wc48[j % P, j // P, combo] = 1.0
    wc48[j % P, j // P, NCC + tg] = 1.0

    return (x3, wc48, s_ii, tg, bt, combo,
            scale_t, scale_b, square_mode, bias_t)


def _epilogue(acc, s_ii, tg, bt, combo, weight_target, weight_batch0,
              scale_t, scale_b, square_mode, bias_t):
    """acc: [120, N] device sums (0:100 combo-Sb, 100:120 target-St)."""
    f = np.float64
    idx = np.arange(N)
    combosum_b = acc[0:NCC].astype(f)          # [100, N]
    classsum_t = acc[NCC:OUT_ROWS].astype(f)   # [20, N]
    rowsum_t = classsum_t.sum(axis=0)
    classsum_b = combosum_b.reshape(NT, NB, N).sum(axis=1)  # [20, N]

    # Replicate the device's fp8-rounded diagonal contributions exactly.
    # Chunk-A queries (q%768 < 512) get ScalarE exp + fp8 RNE; chunk-B
    # queries get the VectorE Schraudolph bit pattern.
    import ml_dtypes
    s32 = s_ii.astype(np.float32)
    diag_act = _e4m3(np.exp(scale_b * s32))
    if square_mode:
        c1 = np.float32(8.0 * scale_b * np.log2(np.e))
        bits = np.round(c1 * s32 + np.float32(56.0)).astype(np.uint8)
        diag_sch = bits.view(ml_dtypes.float8_e4m3fn)
        in_a = (idx % R) < QA
        diag_b_fp8 = np.where(in_a, diag_act, diag_sch)
        diag_b = diag_b_fp8.astype(f)
        diag_t = _bitdouble_f64(diag_b_fp8)
    else:
        diag_b = diag_act.astype(f)
        diag_t = _e4m3(np.exp(scale_t * s32
                              + np.float32(bias_t))).astype(f)

    cnt_t = np.bincount(tg, minlength=NT)
    n_tb = np.zeros((NT, NB), dtype=np.int64)
    np.add.at(n_tb, (tg, bt), 1)

    # ---- target SNN loss ----
    own_t = classsum_t[tg, idx]
    pos_t = own_t - diag_t
    neg_t = rowsum_t - own_t
    cnt_pos = cnt_t[tg]
    cnt_neg = N - cnt_pos
    valid_t = (cnt_pos >= 2) & (cnt_neg >= 1)
    pos_s = np.where(valid_t, pos_t, 1.0)
    neg_s = np.where(valid_t, neg_t, 1.0)
    loss_i = -np.log(pos_s / (pos_s + neg_s))
    lsum = np.bincount(tg, weights=np.where(valid_t, loss_i, 0.0),
                       minlength=NT)
    vcnt = np.bincount(tg, weights=valid_t.astype(f), minlength=NT)
    mean = lsum / np.maximum(vcnt, 1.0)
    wt_w = np.asarray(weight_target).astype(f)
    loss_target = np.where(vcnt > 0, mean * wt_w, 0.0).sum()

    # ---- batch-effect loss ----
    own_tb = combosum_b[combo, idx]
    samet = classsum_b[tg, idx]
    pos_b = own_tb - diag_b
    neg_b = samet - own_tb
    cnt_pos_b = n_tb[tg, bt]
    cnt_neg_b = cnt_t[tg] - cnt_pos_b
    valid_b = (cnt_pos_b >= 2) & (cnt_neg_b >= 1)
    pos_bs = np.where(valid_b, pos_b, 1.0)
    neg_bs = np.where(valid_b, neg_b, 1.0)
    loss_bi = -np.log(pos_bs / (pos_bs + neg_bs))
    inv = np.where(valid_b, 1.0 / np.where(valid_b, loss_bi, 1.0), 0.0)
    lsum_b = np.bincount(bt, weights=inv, minlength=NB)
    vcnt_b = np.bincount(bt, weights=valid_b.astype(f), minlength=NB)
    mean_b = lsum_b / np.maximum(vcnt_b, 1.0)
    wb_w = np.asarray(weight_batch0).astype(f)
    loss_batch = np.where(vcnt_b > 0, mean_b * wb_w, 0.0).sum()

    return np.float32(0.9 * loss_target + 0.1 * loss_batch)


def _run_with_retry(nc, in_maps, core_ids, attempts=3):
    import time as _time

    from concourse.bass_utils import run_bass_kernel_spmd

    for i in range(attempts):
        try:
            return run_bass_kernel_spmd(nc, in_maps, core_ids)
        except Exception:
            if i == attempts - 1:
                raise
            _time.sleep(90)  # transient NRT device errors clear after a pause


def kernel(input, temperature, weight_target, weight_batch0, targets, batch0):
    global LAST_RESULT

    (x3, wc48, s_ii, tg, bt, combo,
     scale_t, scale_b, square_mode, bias_t) = _host_prep(
        input, temperature, targets, batch0)

    _patch_ldw_opt()
    nc = _get_compiled(scale_t, scale_b, square_mode, bias_t)

    in_maps = []
    for c in range(NCORES):
        kroll = 6 * c  # R/P key tiles
        in_maps.append({
            "xnt": np.ascontiguousarray(np.roll(x3, -c * R, axis=1)),
            "wcp": _e4m3(np.roll(wc48, -kroll, axis=1)
                         .reshape(P, NPAIR * 2 * WC)),
        })
    LAST_RESULT = _run_with_retry(nc, in_maps, list(range(NCORES)))
    acc = np.concatenate(
        [LAST_RESULT.results[c]["out"] for c in range(NCORES)], axis=1
    )  # [120, N]

    return _epilogue(acc, s_ii, tg, bt, combo, weight_target, weight_batch0,
                     scale_t, scale_b, square_mode, bias_t)
